# revision 48
# baseline (speedup 1.0000x reference)
"""Trainium2 Bass kernel for nn_Attention (non-local-attention block + sync BN).

Computation per batch element b (B=8, C_IN=256, C_OUT=128, N=4096):
    theta = theta_w @ x + theta_b          [128, 4096]
    phi   = phi_w @ x + phi_b              [128, 4096]
    g     = g_w @ x + g_b                  [128, 4096]
    f     = theta^T @ phi / N              [4096, 4096]  -- never materialized:
    y     = Bm^T @ theta / N  with  Bm = phi @ g^T   [128, 128]
    w_y   = W_w @ y  (+ W_b, cancels in BN)[256, 4096]
    out   = BN(w_y) * gamma + beta + x     (BN stats over all (B, N) -> AllGather)

Design (v3).  Measured on this fleet: the PE is power-throttled to ~50%
utilization for most of the kernel, and the collectives firmware imposes a
~40-50us startup barrier plus ~11us first-collective start latency that no
kernel structure can avoid (hand-rolled SBUF remote-DMA exchange crashes
the devices in this axon-virtualized environment).  The critical path is
therefore: startup barrier -> AllReduce -> stats-dependent tail; all the
compute (which finishes by ~42us) hides under the barrier.

 * phi^T / g^T come from transposed convs: stationary = x 128-col block,
   moving = [phi_w_h | g_w_h] packed [128, 256] -> one PSUM [128n, 512]
   holds TWO m-blocks; conv biases ride the PSUM->SBUF drain as a DVE
   broadcast add (cheaper than K=1 bias matmuls under the PE throttle).
 * B accumulates in a single PSUM bank lagging the tr-conv drains;
   y = B^T theta and the W projection pipeline right behind.
 * BN stats via DVE bn_stats/bn_aggr on the bf16 wy tiles (mean+var in one
   pass, no separate square pass), pre-scaled by 1/B so one tiny [128,4]
   fp32 AllReduce yields the global mean / E[w^2] directly.
 * Everything is bf16 (inputs, residual, output; fp32 PSUM accumulation);
   the fp32 x load is dropped entirely and the output is upcast on host.
"""

import contextlib

import numpy as np
import ml_dtypes

import concourse.bass as bass  # noqa: F401  (registers engines)
import concourse.tile as tile
from concourse import bacc, mybir
from concourse import bass_utils

N_CORES = 8
USE_RDMA = False      # hand-rolled remote-DMA allreduce (crashes this env)
USE_ALLGATHER = True  # AllGather+local sum vs AllReduce for the stats sync
RDMA_ROUNDS = 1
B, C_IN, C_OUT, N = 8, 256, 128, 4096
P = 128
NCH = N // 512    # 8 column chunks of 512
MCH = N // 128    # 32 m-blocks of 128
BLAG = 6          # B-matmul lag (m-blocks) behind tr-conv drains
BN_EPS = 1e-5

F32 = mybir.dt.float32
BF16 = mybir.dt.bfloat16
AF = mybir.ActivationFunctionType
ALU = mybir.AluOpType
AX = mybir.AxisListType


def _build_module():
    nc = bacc.Bacc("TRN2", target_bir_lowering=False, debug=False,
                   enable_asserts=False, num_devices=N_CORES)

    x16 = nc.dram_tensor("x16", [C_IN, N], BF16, kind="ExternalInput").ap()
    # wpack cols: thw0 thw1 | pg0 (phw0|gw0) | pg1 (phw1|gw1) | WwA WwB
    wpack = nc.dram_tensor("wpack", [P, 1024], BF16, kind="ExternalInput").ap()
    # bpack cols: thb(1) gam(2) bet(2) pgb(512: [bphi|bg|bphi|bg] bcast rows)
    bpack = nc.dram_tensor("bpack", [P, 517], F32, kind="ExternalInput").ap()
    out16 = nc.dram_tensor("out16", [C_IN, N], BF16, kind="ExternalOutput").ap()

    with contextlib.ExitStack() as ctx:
        tc = ctx.enter_context(tile.TileContext(nc))
        pp = ctx.enter_context(tc.tile_pool(name="persist", bufs=1))
        ysb = ctx.enter_context(tc.tile_pool(name="ysb", bufs=3))
        op = ctx.enter_context(tc.tile_pool(name="outp", bufs=4))
        ps_cv = ctx.enter_context(tc.tile_pool(name="pscv", bufs=4, space="PSUM"))
        ps_tr = ctx.enter_context(tc.tile_pool(name="pstr", bufs=3, space="PSUM"))
        ps_b = ctx.enter_context(tc.tile_pool(name="psb", bufs=1, space="PSUM"))
        dram = ctx.enter_context(tc.tile_pool(name="dram", bufs=1, space="DRAM"))

        # ---- persistent SBUF tensors ----
        x16h = [pp.tile([P, N], BF16, tag=f"x16_{h}", name=f"x16_{h}")
                for h in range(2)]
        th_t = pp.tile([P, N], BF16, tag="th")
        pg_t = pp.tile([P, 2 * N], BF16, tag="pg")   # [phi^T|g^T] 32 x [128,256]
        wy_t = [pp.tile([P, N], BF16, tag=f"wy{h}", name=f"wy{h}") for h in range(2)]
        bst_t = [pp.tile([P, 6 * NCH], F32, tag=f"bst{h}", name=f"bst{h}")
                 for h in range(2)]                  # bn_stats 6-tuples per chunk

        wp_t = pp.tile([P, 1024], BF16, tag="wp")
        bp_t = pp.tile([P, 517], F32, tag="bp")
        eps_t = pp.tile([P, 1], F32, tag="eps")
        nc.gpsimd.memset(eps_t[:], BN_EPS)
        warm_t = pp.tile([P, 1], F32, tag="warm")

        def cs(i, w):  # column slice helper
            return slice(i * w, (i + 1) * w)

        # weight DMAs first (small), then x16 512-col chunks, h0 on the sync
        # ring and h1 on the scalar ring so the first j-group unblocks ASAP
        nc.sync.dma_start(wp_t[:], wpack[:, :])
        nc.scalar.dma_start(bp_t[:], bpack[:, :])
        for q in range(NCH):
            nc.sync.dma_start(x16h[0][:, cs(q, 512)], x16[0:P, cs(q, 512)])
            nc.scalar.dma_start(x16h[1][:, cs(q, 512)], x16[P:2 * P, cs(q, 512)])
        thw_t = [wp_t[:, cs(k, P)] for k in range(2)]
        pg_w = [wp_t[:, cs(1 + k, 256)] for k in range(2)]   # cols 256:512, 512:768
        Ww_h = [wp_t[:, cs(6 + h, P)] for h in range(2)]
        thb_t = bp_t[:, 0:1]
        gam_t = bp_t[:, 1:3]
        bet_t = bp_t[:, 3:5]
        pgb_t = bp_t[:, 5:517]

        # cross-core BN-stats sync: hand-rolled 3-round hypercube allreduce
        # over SBUF-to-SBUF remote DMA.  This avoids collective_compute
        # entirely -- the CC firmware path costs a 35-50us startup barrier
        # plus >10us per op, while peer (rank XOR 2^k) exchanges of a 2KB
        # tile are a few us total and fully SPMD-symmetric (relative dests).
        if USE_RDMA:
            sem_r = nc.alloc_semaphore("rdma_r")
            sem_l = nc.alloc_semaphore("rdma_l")
            nc.gpsimd.sem_clear(sem_r)
            nc.gpsimd.sem_clear(sem_l)
            land_t = [pp.tile([P, 4], F32, tag=f"land{r}", name=f"land{r}")
                      for r in range(3)]

        # ---- phase 1: theta conv + transposed phi|g convs + B accumulation.
        # One continuous PE stream; drains trail on ACT (theta) and ACT/DVE
        # (tr-conv pairs). theta_w/theta_b carry the 1/N factor (host).
        b_ps = ps_b.tile([P, P], F32, tag="b", name="b_ps")

        def emit_b(mb):
            nc.tensor.matmul(b_ps[:], pg_t[:, mb * 256:mb * 256 + 128],
                             pg_t[:, mb * 256 + 128:mb * 256 + 256],
                             start=(mb == 0), stop=(mb == MCH - 1))

        for j in range(NCH):
            ps = ps_cv.tile([P, 512], F32, tag="cv", name="ps_th")
            nc.tensor.matmul(ps[:], thw_t[0], x16h[0][:, cs(j, 512)],
                             start=True, stop=False)
            nc.tensor.matmul(ps[:], thw_t[1], x16h[1][:, cs(j, 512)],
                             start=False, stop=True)
            nc.scalar.activation(th_t[:, cs(j, 512)], ps[:], AF.Identity,
                                 bias=thb_t)
            for g in range(2):           # two 2-m-block groups per j
                m0 = 4 * j + 2 * g
                tp = ps_tr.tile([P, 512], F32, tag="tr", name="ps_tr")
                for t in range(2):       # m-blocks m0, m0+1
                    m = m0 + t
                    nc.tensor.matmul(tp[:, cs(t, 256)],
                                     x16h[0][:, cs(m, P)], pg_w[0],
                                     start=True, stop=False,
                                     skip_group_check=True)
                    nc.tensor.matmul(tp[:, cs(t, 256)],
                                     x16h[1][:, cs(m, P)], pg_w[1],
                                     start=False, stop=(t == 1),
                                     skip_group_check=True)
                # conv biases ride on the PSUM drain (DVE broadcast add):
                # the PE is power-throttled to ~50% util, so bias columns
                # are cheaper on DVE than as K=1 matmuls
                nc.vector.tensor_tensor(pg_t[:, m0 * 256:m0 * 256 + 512],
                                        tp[:], pgb_t, op=ALU.add)
                mb0 = m0 - BLAG
                for mb in (mb0, mb0 + 1):
                    if 0 <= mb < MCH - BLAG:
                        emit_b(mb)
        nc.scalar.activation(warm_t[:], eps_t[:], AF.Sqrt)  # preload ACT table
        for mb in range(MCH - BLAG, MCH):
            emit_b(mb)
        b_sb = pp.tile([P, P], BF16, tag="b_sb")
        nc.vector.tensor_copy(b_sb[:], b_ps[:])

        # ---- phase 2: y = B^T theta, then W projection, W lagging one chunk;
        # bn_stats on DVE reads the W PSUM directly so the stats path does
        # not serialize behind the ACT wy drains (the drain is only needed
        # for the post-collective affine)
        def emit_w_block(j, y_sb):
            for h in range(2):
                w_ps = ps_cv.tile([P, 512], F32, tag="cv", name="ps_w")
                nc.tensor.matmul(w_ps[:], Ww_h[h], y_sb[:],
                                 start=True, stop=True)
                nc.vector.bn_stats(bst_t[h][:, cs(j, 6)], w_ps[:])
                nc.scalar.activation(wy_t[h][:, cs(j, 512)], w_ps[:], AF.Copy)

        y_sbs = []
        for j in range(NCH):
            y_ps = ps_tr.tile([P, 512], F32, tag="tr", name="y_ps")
            nc.tensor.matmul(y_ps[:], b_sb[:], th_t[:, cs(j, 512)],
                             start=True, stop=True)
            y_sb = ysb.tile([P, 512], BF16, tag="y_sb", name="y_sb")
            if j % 2 == 0:
                nc.scalar.activation(y_sb[:], y_ps[:], AF.Copy)
            else:
                nc.vector.tensor_copy(y_sb[:], y_ps[:])
            y_sbs.append(y_sb)
            if j >= 1:
                emit_w_block(j - 1, y_sbs[j - 1])
        emit_w_block(NCH - 1, y_sbs[NCH - 1])

        # ---- BN stats: bn_aggr per half -> sums, AllReduce, global affine
        mv = pp.tile([P, 4], F32, tag="mv")          # [m0 v0 | m1 v1]
        for h in range(2):
            nc.vector.bn_aggr(mv[:, cs(h, 2)], bst_t[h][:, :])
        s4 = pp.tile([P, 4], F32, tag="s4")          # [S1_0 S1_1 S2_0 S2_1]
        msq = pp.tile([P, 2], F32, tag="msq")
        # scaled by 1/B so the post-AllReduce sums are directly the global
        # mean and E[w^2] (the 1/(B*N) normalization is pre-folded here,
        # where there is slack before the collective):
        # S1 = m/B ; S2 = (v + m^2)/B
        invb = 1.0 / B
        # h0 chain on DVE, h1 chain partly on ACT -- shortens the serial
        # stats path between the last W chunk and the collective trigger
        nc.scalar.activation(s4[:, 1:2], mv[:, 2:3], AF.Copy, scale=invb)
        nc.scalar.square(msq[:, 1:2], mv[:, 2:3])
        nc.vector.tensor_scalar_mul(s4[:, 0:1], mv[:, 0:1], invb)
        nc.vector.tensor_tensor(msq[:, 0:1], mv[:, 0:1], mv[:, 0:1], op=ALU.mult)
        nc.vector.tensor_tensor(msq[:, 0:1], msq[:, 0:1], mv[:, 1:2], op=ALU.add)
        nc.vector.tensor_tensor(msq[:, 1:2], msq[:, 1:2], mv[:, 3:4], op=ALU.add)
        nc.vector.tensor_scalar_mul(s4[:, 2:3], msq[:, 0:1], invb)
        nc.vector.tensor_scalar_mul(s4[:, 3:4], msq[:, 1:2], invb)
        rdma_waits = []
        if USE_RDMA:
            part_t = [s4,
                      pp.tile([P, 4], F32, tag="part1", name="part1"),
                      pp.tile([P, 4], F32, tag="part2", name="part2"),
                      pp.tile([P, 4], F32, tag="g4", name="g4")]
            rdests = [
                [(0, 1)] + [None] * 7,                       # XOR 1
                [(0, 2)] + [None] * 7,                       # XOR 2
                [None] * 4 + [(0, 4)] + [None] * 3,          # XOR 4 (D2D slot)
            ]
            # The adds' waits on the remote-landing semaphore are attached
            # AFTER tile scheduling (the single-core scheduling sim cannot
            # model peer increments and would report a deadlock).
            for r in range(RDMA_ROUNDS):
                nc.gpsimd.remote_dma_broadcast(
                    land_t[r][:], part_t[r][:],
                    remote_sem=sem_r, local_sem=sem_l, rdests=rdests[r])
                nc.gpsimd.trigger_dma(count=None)
                add_i = nc.vector.tensor_tensor(part_t[r + 1][:],
                                                part_t[r][:],
                                                land_t[r][:], op=ALU.add)
                rdma_waits.append((add_i, 2 * (r + 1)))
            g4 = part_t[RDMA_ROUNDS]
        elif USE_ALLGATHER:
            in_b = dram.tile([P, 4], F32)
            out_b = dram.tile([P * N_CORES, 4], F32)
            nc.sync.dma_start(in_b[:], s4[:])
            nc.gpsimd.collective_compute(
                "AllGather", ALU.bypass,
                replica_groups=[list(range(N_CORES))],
                ins=[in_b.opt()], outs=[out_b.opt()],
            )
            # readback as 8 contiguous [128,4] block loads (the single
            # strided gather costs ~6us in descriptor processing)
            g32 = pp.tile([P, 32], F32, tag="g32")
            rb_eng = [nc.sync, nc.scalar, nc.sync, nc.scalar,
                      nc.sync, nc.scalar, nc.sync, nc.scalar]
            for r in range(N_CORES):
                rb_eng[r].dma_start(g32[:, cs(r, 4)], out_b[r * P:(r + 1) * P, :])
            g4 = pp.tile([P, 4], F32, tag="g4", name="g4")
            nc.vector.reduce_sum(g4[:], g32[:].rearrange("p (r c) -> p c r",
                                                         r=N_CORES), axis=AX.X)
        else:
            in_b = dram.tile([P, 4], F32)
            out_b = dram.tile([P, 4], F32)
            nc.sync.dma_start(in_b[:], s4[:])
            nc.gpsimd.collective_compute(
                "AllReduce", ALU.add,
                replica_groups=[list(range(N_CORES))],
                ins=[in_b.opt()], outs=[out_b.opt()],
            )
            g4 = pp.tile([P, 4], F32, tag="g4", name="g4")
            nc.sync.dma_start(g4[:], out_b[:])

        var = pp.tile([P, 2], F32, tag="var")
        tmp = pp.tile([P, 2], F32, tag="tmp")
        sd = pp.tile([P, 2], F32, tag="sd")
        scl = pp.tile([P, 2], F32, tag="scl")
        bia = pp.tile([P, 2], F32, tag="bia")
        mn = g4[:, 0:2]                          # global mean (pre-scaled)
        nc.vector.tensor_mul(tmp[:], mn, mn)
        nc.vector.tensor_sub(var[:], g4[:, 2:4], tmp[:])
        nc.scalar.activation(sd[:], var[:], AF.Sqrt, bias=eps_t[:, 0:1])
        rstd = pp.tile([P, 2], F32, tag="rstd")
        nc.vector.reciprocal(rstd[:], sd[:])
        nc.vector.tensor_mul(scl[:], rstd[:], gam_t)
        nc.vector.tensor_mul(tmp[:], mn, scl[:])
        nc.vector.tensor_sub(bia[:], bet_t, tmp[:])

        # ---- normalize + residual + store (all bf16): [128,2048] chunks to
        # amortize per-op fixed costs; affine split ACT/DVE (3:1), adds on
        # DVE, each chunk stored as two [128,1024] halves on both HWDGE
        # rings.  This tail is the only stats-dependent work, so it alone
        # sits after the AllReduce.
        for idx in range(4):
            h, j = divmod(idx, 2)
            o1 = op.tile([P, 2048], BF16, tag="o1", name="o1")
            o2 = op.tile([P, 2048], BF16, tag="o2", name="o2")
            if idx < 3:
                nc.scalar.activation(o1[:], wy_t[h][:, cs(j, 2048)],
                                     AF.Identity, bias=bia[:, h:h + 1],
                                     scale=scl[:, h:h + 1])
            else:
                nc.vector.tensor_scalar(o1[:], wy_t[h][:, cs(j, 2048)],
                                        scl[:, h:h + 1], bia[:, h:h + 1],
                                        op0=ALU.mult, op1=ALU.add)
            nc.vector.tensor_tensor(o2[:], o1[:], x16h[h][:, cs(j, 2048)],
                                    op=ALU.add)
            nc.sync.dma_start(out16[h * P:(h + 1) * P,
                                    2048 * j:2048 * j + 1024], o2[:, 0:1024])
            nc.scalar.dma_start(out16[h * P:(h + 1) * P,
                                      2048 * j + 1024:2048 * (j + 1)],
                                o2[:, 1024:2048])

    for add_i, thr in rdma_waits:
        add_i.wait_op(sem_r, thr, "sem-ge", check=False)
    nc.compile()
    return nc


_CACHE = {}


def _get_module():
    if "nc" not in _CACHE:
        _CACHE["nc"] = _build_module()
    return _CACHE["nc"]


def _prep_in_maps(x, g_w, g_b, theta_w, theta_b, phi_w, phi_b, W_w, W_b,
                  bn_gamma, bn_beta):
    bf = ml_dtypes.bfloat16
    f32 = np.float32
    thwT = (theta_w.T / N).astype(bf)
    phwT = phi_w.T.astype(bf)
    gwT = g_w.T.astype(bf)
    WwT = W_w.T.astype(bf)
    wpack = np.concatenate(
        [thwT[0:P], thwT[P:2 * P],
         phwT[0:P], gwT[0:P], phwT[P:2 * P], gwT[P:2 * P],
         WwT[:, 0:P], WwT[:, P:2 * P]], axis=1)
    pgb = np.broadcast_to(
        np.concatenate([phi_b, g_b, phi_b, g_b])[None, :], (P, 512))
    bpack = np.concatenate(
        [(theta_b / N).reshape(P, 1).astype(f32),
         bn_gamma.reshape(2, P).T.astype(f32),
         bn_beta.reshape(2, P).T.astype(f32),
         pgb.astype(f32)], axis=1)
    shared = {
        "wpack": np.ascontiguousarray(wpack),
        "bpack": np.ascontiguousarray(bpack),
    }
    x16 = np.ascontiguousarray(x.astype(bf))
    in_maps = []
    for i in range(N_CORES):
        m = dict(shared)
        m["x16"] = x16[i]
        in_maps.append(m)
    return in_maps


def _run(inputs, trace=False, trace_cores=None):
    nc = _get_module()
    in_maps = _prep_in_maps(**inputs)
    res = bass_utils.run_bass_kernel_spmd(
        nc, in_maps, core_ids=list(range(N_CORES)),
        trace=trace, trace_cores=trace_cores,
    )
    out = np.stack([res.results[i]["out16"] for i in range(N_CORES)], axis=0)
    return out.astype(np.float32), res


def kernel(**inputs) -> np.ndarray:
    out, _ = _run(inputs, trace=False)
    return out


# revision 52
# speedup vs baseline: 1.0382x; 1.0382x over previous
"""Trainium2 Bass kernel for nn_Attention (non-local-attention block + sync BN).

Computation per batch element b (B=8, C_IN=256, C_OUT=128, N=4096):
    theta = theta_w @ x + theta_b          [128, 4096]
    phi   = phi_w @ x + phi_b              [128, 4096]
    g     = g_w @ x + g_b                  [128, 4096]
    f     = theta^T @ phi / N              [4096, 4096]  -- never materialized:
    y     = Bm^T @ theta / N  with  Bm = phi @ g^T   [128, 128]
    w_y   = W_w @ y  (+ W_b, cancels in BN)[256, 4096]
    out   = BN(w_y) * gamma + beta + x     (BN stats over all (B, N) -> AllGather)

Design (v3).  Measured on this fleet: the PE is power-throttled to ~50%
utilization for most of the kernel, and the collectives firmware imposes a
~40-50us startup barrier plus ~11us first-collective start latency that no
kernel structure can avoid (hand-rolled SBUF remote-DMA exchange crashes
the devices in this axon-virtualized environment).  The critical path is
therefore: startup barrier -> AllReduce -> stats-dependent tail; all the
compute (which finishes by ~42us) hides under the barrier.

 * phi^T / g^T come from transposed convs: stationary = x 128-col block,
   moving = [phi_w_h | g_w_h] packed [128, 256] -> one PSUM [128n, 512]
   holds TWO m-blocks; conv biases ride the PSUM->SBUF drain as a DVE
   broadcast add (cheaper than K=1 bias matmuls under the PE throttle).
 * B accumulates in a single PSUM bank lagging the tr-conv drains;
   y = B^T theta and the W projection pipeline right behind.
 * BN stats via DVE bn_stats/bn_aggr on the bf16 wy tiles (mean+var in one
   pass, no separate square pass), pre-scaled by 1/B so one tiny [128,4]
   fp32 AllReduce yields the global mean / E[w^2] directly.
 * Everything is bf16 (inputs, residual, output; fp32 PSUM accumulation);
   the fp32 x load is dropped entirely and the output is upcast on host.
"""

import contextlib

import numpy as np
import ml_dtypes

import concourse.bass as bass  # noqa: F401  (registers engines)
import concourse.tile as tile
from concourse import bacc, mybir
from concourse import bass_utils

N_CORES = 8
USE_RDMA = False      # hand-rolled remote-DMA allreduce (crashes this env)
USE_ALLGATHER = True  # AllGather+local sum vs AllReduce for the stats sync
RDMA_ROUNDS = 1
B, C_IN, C_OUT, N = 8, 256, 128, 4096
P = 128
NCH = N // 512    # 8 column chunks of 512
MCH = N // 128    # 32 m-blocks of 128
BLAG = 6          # B-matmul lag (m-blocks) behind tr-conv drains
BN_EPS = 1e-5

F32 = mybir.dt.float32
BF16 = mybir.dt.bfloat16
AF = mybir.ActivationFunctionType
ALU = mybir.AluOpType
AX = mybir.AxisListType


def _build_module():
    nc = bacc.Bacc("TRN2", target_bir_lowering=False, debug=False,
                   enable_asserts=False, num_devices=N_CORES)

    x16 = nc.dram_tensor("x16", [C_IN, N], BF16, kind="ExternalInput").ap()
    # wpack cols: thw0 thw1 | pg0 (phw0|gw0) | pg1 (phw1|gw1) | WwA WwB
    wpack = nc.dram_tensor("wpack", [P, 1024], BF16, kind="ExternalInput").ap()
    # bpack cols: thb(1) gam(2) bet(2) pgb(512: [bphi|bg|bphi|bg] bcast rows)
    bpack = nc.dram_tensor("bpack", [P, 517], F32, kind="ExternalInput").ap()
    out16 = nc.dram_tensor("out16", [C_IN, N], BF16, kind="ExternalOutput").ap()

    with contextlib.ExitStack() as ctx:
        tc = ctx.enter_context(tile.TileContext(nc))
        pp = ctx.enter_context(tc.tile_pool(name="persist", bufs=1))
        op = ctx.enter_context(tc.tile_pool(name="outp", bufs=4))
        ps_cv = ctx.enter_context(tc.tile_pool(name="pscv", bufs=3, space="PSUM"))
        ps_tr = ctx.enter_context(tc.tile_pool(name="pstr", bufs=3, space="PSUM"))
        ps_b = ctx.enter_context(tc.tile_pool(name="psb", bufs=1, space="PSUM"))
        dram = ctx.enter_context(tc.tile_pool(name="dram", bufs=1, space="DRAM"))

        # ---- persistent SBUF tensors ----
        x16h = [pp.tile([P, N], BF16, tag=f"x16_{h}", name=f"x16_{h}")
                for h in range(2)]
        th_t = pp.tile([P, N], BF16, tag="th")
        pg_t = pp.tile([P, 2 * N], BF16, tag="pg")   # [phi^T|g^T] 32 x [128,256]
        wy_t = [pp.tile([P, N], BF16, tag=f"wy{h}", name=f"wy{h}") for h in range(2)]
        bst_t = [pp.tile([P, 6 * NCH], F32, tag=f"bst{h}", name=f"bst{h}")
                 for h in range(2)]                  # bn_stats 6-tuples per chunk

        wp_t = pp.tile([P, 1024], BF16, tag="wp")
        bp_t = pp.tile([P, 517], F32, tag="bp")
        eps_t = pp.tile([P, 1], F32, tag="eps")
        nc.gpsimd.memset(eps_t[:], BN_EPS)
        warm_t = pp.tile([P, 1], F32, tag="warm")

        def cs(i, w):  # column slice helper
            return slice(i * w, (i + 1) * w)

        # weight DMAs first (small), then x16 512-col chunks, h0 on the sync
        # ring and h1 on the scalar ring so the first j-group unblocks ASAP
        nc.sync.dma_start(wp_t[:], wpack[:, :])
        nc.scalar.dma_start(bp_t[:], bpack[:, :])
        for q in range(NCH):
            nc.sync.dma_start(x16h[0][:, cs(q, 512)], x16[0:P, cs(q, 512)])
            nc.scalar.dma_start(x16h[1][:, cs(q, 512)], x16[P:2 * P, cs(q, 512)])
        thw_t = [wp_t[:, cs(k, P)] for k in range(2)]
        pg_w = [wp_t[:, cs(1 + k, 256)] for k in range(2)]   # cols 256:512, 512:768
        Ww_h = [wp_t[:, cs(6 + h, P)] for h in range(2)]
        thb_t = bp_t[:, 0:1]
        gam_t = bp_t[:, 1:3]
        bet_t = bp_t[:, 3:5]
        pgb_t = bp_t[:, 5:517]

        # cross-core BN-stats sync: hand-rolled 3-round hypercube allreduce
        # over SBUF-to-SBUF remote DMA.  This avoids collective_compute
        # entirely -- the CC firmware path costs a 35-50us startup barrier
        # plus >10us per op, while peer (rank XOR 2^k) exchanges of a 2KB
        # tile are a few us total and fully SPMD-symmetric (relative dests).
        if USE_RDMA:
            sem_r = nc.alloc_semaphore("rdma_r")
            sem_l = nc.alloc_semaphore("rdma_l")
            nc.gpsimd.sem_clear(sem_r)
            nc.gpsimd.sem_clear(sem_l)
            land_t = [pp.tile([P, 4], F32, tag=f"land{r}", name=f"land{r}")
                      for r in range(3)]

        # ---- phase 1: theta conv + transposed phi|g convs + B accumulation.
        # One continuous PE stream; drains trail on ACT (theta) and ACT/DVE
        # (tr-conv pairs). theta_w/theta_b carry the 1/N factor (host).
        b_ps = ps_b.tile([P, P], F32, tag="b", name="b_ps")

        # accumulates B^T = g @ phi^T (stationary = g^T block): exactly the
        # stationary layout needed for D = B^T-acc @ WwT below
        def emit_b(mb):
            nc.tensor.matmul(b_ps[:], pg_t[:, mb * 256 + 128:mb * 256 + 256],
                             pg_t[:, mb * 256:mb * 256 + 128],
                             start=(mb == 0), stop=(mb == MCH - 1))

        for j in range(NCH):
            ps = ps_cv.tile([P, 512], F32, tag="cv", name="ps_th")
            nc.tensor.matmul(ps[:], thw_t[0], x16h[0][:, cs(j, 512)],
                             start=True, stop=False)
            nc.tensor.matmul(ps[:], thw_t[1], x16h[1][:, cs(j, 512)],
                             start=False, stop=True)
            nc.scalar.activation(th_t[:, cs(j, 512)], ps[:], AF.Identity,
                                 bias=thb_t)
            for g in range(2):           # two 2-m-block groups per j
                m0 = 4 * j + 2 * g
                tp = ps_tr.tile([P, 512], F32, tag="tr", name="ps_tr")
                for t in range(2):       # m-blocks m0, m0+1
                    m = m0 + t
                    nc.tensor.matmul(tp[:, cs(t, 256)],
                                     x16h[0][:, cs(m, P)], pg_w[0],
                                     start=True, stop=False,
                                     skip_group_check=True)
                    nc.tensor.matmul(tp[:, cs(t, 256)],
                                     x16h[1][:, cs(m, P)], pg_w[1],
                                     start=False, stop=(t == 1),
                                     skip_group_check=True)
                # conv biases ride on the PSUM drain (DVE broadcast add):
                # the PE is power-throttled to ~50% util, so bias columns
                # are cheaper on DVE than as K=1 matmuls
                nc.vector.tensor_tensor(pg_t[:, m0 * 256:m0 * 256 + 512],
                                        tp[:], pgb_t, op=ALU.add)
                mb0 = m0 - BLAG
                for mb in (mb0, mb0 + 1):
                    if 0 <= mb < MCH - BLAG:
                        emit_b(mb)
        nc.scalar.activation(warm_t[:], eps_t[:], AF.Sqrt)  # preload ACT table
        for mb in range(MCH - BLAG, MCH):
            emit_b(mb)
        bt_sb = pp.tile([P, P], BF16, tag="bt_sb")
        nc.vector.tensor_copy(bt_sb[:], b_ps[:])

        # ---- phase 2: fold the y stage into the W projection.  Since y is
        # only consumed by W, wy = Ww y = Ww B^T theta = D^T theta with
        # D = B^T-acc @ WwT  [128, 256] -- one tiny matmul -- and the whole
        # former y phase (8 matmuls + 8 drains) disappears.  bn_stats on DVE
        # reads the W PSUM directly so the stats path does not serialize
        # behind the ACT wy drains (those are only needed for the affine).
        d_ps = ps_b.tile([P, 256], F32, tag="d", name="d_ps")
        nc.tensor.matmul(d_ps[:], bt_sb[:], wp_t[:, 768:1024],
                         start=True, stop=True)
        d_sb = pp.tile([P, 256], BF16, tag="d_sb")
        nc.vector.tensor_copy(d_sb[:], d_ps[:])

        for j in range(NCH):
            for h in range(2):
                w_ps = ps_cv.tile([P, 512], F32, tag="cv", name="ps_w")
                nc.tensor.matmul(w_ps[:], d_sb[:, cs(h, P)],
                                 th_t[:, cs(j, 512)], start=True, stop=True)
                nc.vector.bn_stats(bst_t[h][:, cs(j, 6)], w_ps[:])
                nc.scalar.activation(wy_t[h][:, cs(j, 512)], w_ps[:], AF.Copy)

        # ---- BN stats: bn_aggr per half -> sums, AllReduce, global affine
        mv = pp.tile([P, 4], F32, tag="mv")          # [m0 v0 | m1 v1]
        for h in range(2):
            nc.vector.bn_aggr(mv[:, cs(h, 2)], bst_t[h][:, :])
        s4 = pp.tile([P, 4], F32, tag="s4")          # [S1_0 S1_1 S2_0 S2_1]
        msq = pp.tile([P, 2], F32, tag="msq")
        # scaled by 1/B so the post-AllReduce sums are directly the global
        # mean and E[w^2] (the 1/(B*N) normalization is pre-folded here,
        # where there is slack before the collective):
        # S1 = m/B ; S2 = (v + m^2)/B
        invb = 1.0 / B
        # h0 chain on DVE, h1 chain partly on ACT -- shortens the serial
        # stats path between the last W chunk and the collective trigger
        nc.scalar.activation(s4[:, 1:2], mv[:, 2:3], AF.Copy, scale=invb)
        nc.scalar.square(msq[:, 1:2], mv[:, 2:3])
        nc.vector.tensor_scalar_mul(s4[:, 0:1], mv[:, 0:1], invb)
        nc.vector.tensor_tensor(msq[:, 0:1], mv[:, 0:1], mv[:, 0:1], op=ALU.mult)
        nc.vector.tensor_tensor(msq[:, 0:1], msq[:, 0:1], mv[:, 1:2], op=ALU.add)
        nc.vector.tensor_tensor(msq[:, 1:2], msq[:, 1:2], mv[:, 3:4], op=ALU.add)
        nc.vector.tensor_scalar_mul(s4[:, 2:3], msq[:, 0:1], invb)
        nc.vector.tensor_scalar_mul(s4[:, 3:4], msq[:, 1:2], invb)
        rdma_waits = []
        if USE_RDMA:
            part_t = [s4,
                      pp.tile([P, 4], F32, tag="part1", name="part1"),
                      pp.tile([P, 4], F32, tag="part2", name="part2"),
                      pp.tile([P, 4], F32, tag="g4", name="g4")]
            rdests = [
                [(0, 1)] + [None] * 7,                       # XOR 1
                [(0, 2)] + [None] * 7,                       # XOR 2
                [None] * 4 + [(0, 4)] + [None] * 3,          # XOR 4 (D2D slot)
            ]
            # The adds' waits on the remote-landing semaphore are attached
            # AFTER tile scheduling (the single-core scheduling sim cannot
            # model peer increments and would report a deadlock).
            for r in range(RDMA_ROUNDS):
                nc.gpsimd.remote_dma_broadcast(
                    land_t[r][:], part_t[r][:],
                    remote_sem=sem_r, local_sem=sem_l, rdests=rdests[r])
                nc.gpsimd.trigger_dma(count=None)
                add_i = nc.vector.tensor_tensor(part_t[r + 1][:],
                                                part_t[r][:],
                                                land_t[r][:], op=ALU.add)
                rdma_waits.append((add_i, 2 * (r + 1)))
            g4 = part_t[RDMA_ROUNDS]
        elif USE_ALLGATHER:
            in_b = dram.tile([P, 4], F32)
            out_b = dram.tile([P * N_CORES, 4], F32)
            nc.sync.dma_start(in_b[:], s4[:])
            nc.gpsimd.collective_compute(
                "AllGather", ALU.bypass,
                replica_groups=[list(range(N_CORES))],
                ins=[in_b.opt()], outs=[out_b.opt()],
            )
            # readback as 8 contiguous [128,4] block loads (the single
            # strided gather costs ~6us in descriptor processing)
            g32 = pp.tile([P, 32], F32, tag="g32")
            rb_eng = [nc.sync, nc.scalar, nc.sync, nc.scalar,
                      nc.sync, nc.scalar, nc.sync, nc.scalar]
            for r in range(N_CORES):
                rb_eng[r].dma_start(g32[:, cs(r, 4)], out_b[r * P:(r + 1) * P, :])
            g4 = pp.tile([P, 4], F32, tag="g4", name="g4")
            nc.vector.reduce_sum(g4[:], g32[:].rearrange("p (r c) -> p c r",
                                                         r=N_CORES), axis=AX.X)
        else:
            in_b = dram.tile([P, 4], F32)
            out_b = dram.tile([P, 4], F32)
            nc.sync.dma_start(in_b[:], s4[:])
            nc.gpsimd.collective_compute(
                "AllReduce", ALU.add,
                replica_groups=[list(range(N_CORES))],
                ins=[in_b.opt()], outs=[out_b.opt()],
            )
            g4 = pp.tile([P, 4], F32, tag="g4", name="g4")
            nc.sync.dma_start(g4[:], out_b[:])

        var = pp.tile([P, 2], F32, tag="var")
        tmp = pp.tile([P, 2], F32, tag="tmp")
        sd = pp.tile([P, 2], F32, tag="sd")
        scl = pp.tile([P, 2], F32, tag="scl")
        bia = pp.tile([P, 2], F32, tag="bia")
        mn = g4[:, 0:2]                          # global mean (pre-scaled)
        nc.vector.tensor_mul(tmp[:], mn, mn)
        nc.vector.tensor_sub(var[:], g4[:, 2:4], tmp[:])
        nc.scalar.activation(sd[:], var[:], AF.Sqrt, bias=eps_t[:, 0:1])
        rstd = pp.tile([P, 2], F32, tag="rstd")
        nc.vector.reciprocal(rstd[:], sd[:])
        nc.vector.tensor_mul(scl[:], rstd[:], gam_t)
        nc.vector.tensor_mul(tmp[:], mn, scl[:])
        nc.vector.tensor_sub(bia[:], bet_t, tmp[:])

        # ---- normalize + residual + store (all bf16): [128,2048] chunks to
        # amortize per-op fixed costs; affine split ACT/DVE (3:1), adds on
        # DVE, each chunk stored as two [128,1024] halves on both HWDGE
        # rings.  This tail is the only stats-dependent work, so it alone
        # sits after the AllReduce.
        for idx in range(4):
            h, j = divmod(idx, 2)
            o1 = op.tile([P, 2048], BF16, tag="o1", name="o1")
            o2 = op.tile([P, 2048], BF16, tag="o2", name="o2")
            if idx < 3:
                nc.scalar.activation(o1[:], wy_t[h][:, cs(j, 2048)],
                                     AF.Identity, bias=bia[:, h:h + 1],
                                     scale=scl[:, h:h + 1])
            else:
                nc.vector.tensor_scalar(o1[:], wy_t[h][:, cs(j, 2048)],
                                        scl[:, h:h + 1], bia[:, h:h + 1],
                                        op0=ALU.mult, op1=ALU.add)
            nc.vector.tensor_tensor(o2[:], o1[:], x16h[h][:, cs(j, 2048)],
                                    op=ALU.add)
            nc.sync.dma_start(out16[h * P:(h + 1) * P,
                                    2048 * j:2048 * j + 1024], o2[:, 0:1024])
            nc.scalar.dma_start(out16[h * P:(h + 1) * P,
                                      2048 * j + 1024:2048 * (j + 1)],
                                o2[:, 1024:2048])

    for add_i, thr in rdma_waits:
        add_i.wait_op(sem_r, thr, "sem-ge", check=False)
    nc.compile()
    return nc


_CACHE = {}


def _get_module():
    if "nc" not in _CACHE:
        _CACHE["nc"] = _build_module()
    return _CACHE["nc"]


def _prep_in_maps(x, g_w, g_b, theta_w, theta_b, phi_w, phi_b, W_w, W_b,
                  bn_gamma, bn_beta):
    bf = ml_dtypes.bfloat16
    f32 = np.float32
    thwT = (theta_w.T / N).astype(bf)
    phwT = phi_w.T.astype(bf)
    gwT = g_w.T.astype(bf)
    WwT = W_w.T.astype(bf)
    wpack = np.concatenate(
        [thwT[0:P], thwT[P:2 * P],
         phwT[0:P], gwT[0:P], phwT[P:2 * P], gwT[P:2 * P],
         WwT[:, 0:P], WwT[:, P:2 * P]], axis=1)
    pgb = np.broadcast_to(
        np.concatenate([phi_b, g_b, phi_b, g_b])[None, :], (P, 512))
    bpack = np.concatenate(
        [(theta_b / N).reshape(P, 1).astype(f32),
         bn_gamma.reshape(2, P).T.astype(f32),
         bn_beta.reshape(2, P).T.astype(f32),
         pgb.astype(f32)], axis=1)
    shared = {
        "wpack": np.ascontiguousarray(wpack),
        "bpack": np.ascontiguousarray(bpack),
    }
    x16 = np.ascontiguousarray(x.astype(bf))
    in_maps = []
    for i in range(N_CORES):
        m = dict(shared)
        m["x16"] = x16[i]
        in_maps.append(m)
    return in_maps


def _run(inputs, trace=False, trace_cores=None):
    nc = _get_module()
    in_maps = _prep_in_maps(**inputs)
    res = bass_utils.run_bass_kernel_spmd(
        nc, in_maps, core_ids=list(range(N_CORES)),
        trace=trace, trace_cores=trace_cores,
    )
    out = np.stack([res.results[i]["out16"] for i in range(N_CORES)], axis=0)
    return out.astype(np.float32), res


def kernel(**inputs) -> np.ndarray:
    out, _ = _run(inputs, trace=False)
    return out


# revision 54
# speedup vs baseline: 1.0929x; 1.0527x over previous
"""Trainium2 Bass kernel for nn_Attention (non-local-attention block + sync BN).

Computation per batch element b (B=8, C_IN=256, C_OUT=128, N=4096):
    theta = theta_w @ x + theta_b          [128, 4096]
    phi   = phi_w @ x + phi_b              [128, 4096]
    g     = g_w @ x + g_b                  [128, 4096]
    f     = theta^T @ phi / N              [4096, 4096]  -- never materialized:
    y     = Bm^T @ theta / N  with  Bm = phi @ g^T   [128, 128]
    w_y   = W_w @ y  (+ W_b, cancels in BN)[256, 4096]
    out   = BN(w_y) * gamma + beta + x     (BN stats over all (B, N) -> AllGather)

Design (v3).  Measured on this fleet: the PE is power-throttled to ~50%
utilization for most of the kernel, and the collectives firmware imposes a
~40-50us startup barrier plus ~11us first-collective start latency that no
kernel structure can avoid (hand-rolled SBUF remote-DMA exchange crashes
the devices in this axon-virtualized environment).  The critical path is
therefore: startup barrier -> AllReduce -> stats-dependent tail; all the
compute (which finishes by ~42us) hides under the barrier.

 * phi^T / g^T come from transposed convs: stationary = x 128-col block,
   moving = [phi_w_h | g_w_h] packed [128, 256] -> one PSUM [128n, 512]
   holds TWO m-blocks; conv biases ride the PSUM->SBUF drain as a DVE
   broadcast add (cheaper than K=1 bias matmuls under the PE throttle).
 * B accumulates in a single PSUM bank lagging the tr-conv drains;
   y = B^T theta and the W projection pipeline right behind.
 * BN stats via DVE bn_stats/bn_aggr on the bf16 wy tiles (mean+var in one
   pass, no separate square pass), pre-scaled by 1/B so one tiny [128,4]
   fp32 AllReduce yields the global mean / E[w^2] directly.
 * Everything is bf16 (inputs, residual, output; fp32 PSUM accumulation);
   the fp32 x load is dropped entirely and the output is upcast on host.
"""

import contextlib

import numpy as np
import ml_dtypes

import concourse.bass as bass  # noqa: F401  (registers engines)
import concourse.tile as tile
from concourse import bacc, mybir
from concourse import bass_utils

N_CORES = 8
USE_RDMA = False      # hand-rolled remote-DMA allreduce (crashes this env)
USE_ALLGATHER = True  # AllGather+local sum vs AllReduce for the stats sync
RDMA_ROUNDS = 1
B, C_IN, C_OUT, N = 8, 256, 128, 4096
P = 128
NCH = N // 512    # 8 column chunks of 512
MCH = N // 128    # 32 m-blocks of 128
BLAG = 6          # B-matmul lag (m-blocks) behind tr-conv drains
BN_EPS = 1e-5

F32 = mybir.dt.float32
BF16 = mybir.dt.bfloat16
AF = mybir.ActivationFunctionType
ALU = mybir.AluOpType
AX = mybir.AxisListType


def _build_module():
    nc = bacc.Bacc("TRN2", target_bir_lowering=False, debug=False,
                   enable_asserts=False, num_devices=N_CORES)

    x16 = nc.dram_tensor("x16", [C_IN, N], BF16, kind="ExternalInput").ap()
    # wpack cols: thw0 thw1 | pg0 (phw0|gw0) | pg1 (phw1|gw1) | WwA WwB
    wpack = nc.dram_tensor("wpack", [P, 1024], BF16, kind="ExternalInput").ap()
    # bpack cols: thb(1) gam(2) bet(2) pgb(512: [bphi|bg|bphi|bg] bcast rows)
    bpack = nc.dram_tensor("bpack", [P, 517], F32, kind="ExternalInput").ap()
    out16 = nc.dram_tensor("out16", [C_IN, N], BF16, kind="ExternalOutput").ap()

    with contextlib.ExitStack() as ctx:
        tc = ctx.enter_context(tile.TileContext(nc))
        pp = ctx.enter_context(tc.tile_pool(name="persist", bufs=1))
        op = ctx.enter_context(tc.tile_pool(name="outp", bufs=4))
        ps_cv = ctx.enter_context(tc.tile_pool(name="pscv", bufs=4, space="PSUM"))
        ps_tr = ctx.enter_context(tc.tile_pool(name="pstr", bufs=3, space="PSUM"))
        ps_b = ctx.enter_context(tc.tile_pool(name="psb", bufs=1, space="PSUM"))
        dram = ctx.enter_context(tc.tile_pool(name="dram", bufs=1, space="DRAM"))

        # ---- persistent SBUF tensors ----
        x16h = [pp.tile([P, N], BF16, tag=f"x16_{h}", name=f"x16_{h}")
                for h in range(2)]
        th_t = pp.tile([P, N], BF16, tag="th")
        pg_t = pp.tile([P, 2 * N], BF16, tag="pg")   # [phi^T|g^T] 32 x [128,256]
        wy_t = [pp.tile([P, N], BF16, tag=f"wy{h}", name=f"wy{h}") for h in range(2)]
        bst_t = [pp.tile([P, 6 * NCH], F32, tag=f"bst{h}", name=f"bst{h}")
                 for h in range(2)]                  # bn_stats 6-tuples per chunk

        wp_t = pp.tile([P, 1024], BF16, tag="wp")
        bp_t = pp.tile([P, 517], F32, tag="bp")
        eps_t = pp.tile([P, 1], F32, tag="eps")
        nc.gpsimd.memset(eps_t[:], BN_EPS)
        warm_t = pp.tile([P, 1], F32, tag="warm")

        def cs(i, w):  # column slice helper
            return slice(i * w, (i + 1) * w)

        # weight DMAs first (small), then x16 512-col chunks, h0 on the sync
        # ring and h1 on the scalar ring so the first j-group unblocks ASAP
        nc.sync.dma_start(wp_t[:], wpack[:, :])
        nc.scalar.dma_start(bp_t[:], bpack[:, :])
        for q in range(NCH):
            nc.sync.dma_start(x16h[0][:, cs(q, 512)], x16[0:P, cs(q, 512)])
            nc.scalar.dma_start(x16h[1][:, cs(q, 512)], x16[P:2 * P, cs(q, 512)])
        thw_t = [wp_t[:, cs(k, P)] for k in range(2)]
        pg_w = [wp_t[:, cs(1 + k, 256)] for k in range(2)]   # cols 256:512, 512:768
        Ww_h = [wp_t[:, cs(6 + h, P)] for h in range(2)]
        thb_t = bp_t[:, 0:1]
        gam_t = bp_t[:, 1:3]
        bet_t = bp_t[:, 3:5]
        pgb_t = bp_t[:, 5:517]

        # cross-core BN-stats sync: hand-rolled 3-round hypercube allreduce
        # over SBUF-to-SBUF remote DMA.  This avoids collective_compute
        # entirely -- the CC firmware path costs a 35-50us startup barrier
        # plus >10us per op, while peer (rank XOR 2^k) exchanges of a 2KB
        # tile are a few us total and fully SPMD-symmetric (relative dests).
        if USE_RDMA:
            sem_r = nc.alloc_semaphore("rdma_r")
            sem_l = nc.alloc_semaphore("rdma_l")
            nc.gpsimd.sem_clear(sem_r)
            nc.gpsimd.sem_clear(sem_l)
            land_t = [pp.tile([P, 4], F32, tag=f"land{r}", name=f"land{r}")
                      for r in range(3)]

        # ---- phase 1: theta conv + transposed phi|g convs + B accumulation.
        # One continuous PE stream; drains trail on ACT (theta) and ACT/DVE
        # (tr-conv pairs). theta_w/theta_b carry the 1/N factor (host).
        b_ps = ps_b.tile([P, P], F32, tag="b", name="b_ps")

        # accumulates B^T = g @ phi^T (stationary = g^T block): exactly the
        # stationary layout needed for D = B^T-acc @ WwT below
        def emit_b(mb):
            nc.tensor.matmul(b_ps[:], pg_t[:, mb * 256 + 128:mb * 256 + 256],
                             pg_t[:, mb * 256:mb * 256 + 128],
                             start=(mb == 0), stop=(mb == MCH - 1))

        for j in range(NCH):
            ps = ps_cv.tile([P, 512], F32, tag="cv", name="ps_th")
            nc.tensor.matmul(ps[:], thw_t[0], x16h[0][:, cs(j, 512)],
                             start=True, stop=False)
            nc.tensor.matmul(ps[:], thw_t[1], x16h[1][:, cs(j, 512)],
                             start=False, stop=True)
            nc.scalar.activation(th_t[:, cs(j, 512)], ps[:], AF.Identity,
                                 bias=thb_t)
            for g in range(2):           # two 2-m-block groups per j
                m0 = 4 * j + 2 * g
                tp = ps_tr.tile([P, 512], F32, tag="tr", name="ps_tr")
                for t in range(2):       # m-blocks m0, m0+1
                    m = m0 + t
                    nc.tensor.matmul(tp[:, cs(t, 256)],
                                     x16h[0][:, cs(m, P)], pg_w[0],
                                     start=True, stop=False,
                                     skip_group_check=True)
                    nc.tensor.matmul(tp[:, cs(t, 256)],
                                     x16h[1][:, cs(m, P)], pg_w[1],
                                     start=False, stop=(t == 1),
                                     skip_group_check=True)
                # conv biases ride on the PSUM drain (DVE broadcast add):
                # the PE is power-throttled to ~50% util, so bias columns
                # are cheaper on DVE than as K=1 matmuls
                nc.vector.tensor_tensor(pg_t[:, m0 * 256:m0 * 256 + 512],
                                        tp[:], pgb_t, op=ALU.add)
                mb0 = m0 - BLAG
                for mb in (mb0, mb0 + 1):
                    if 0 <= mb < MCH - BLAG:
                        emit_b(mb)
        nc.scalar.activation(warm_t[:], eps_t[:], AF.Sqrt)  # preload ACT table
        for mb in range(MCH - BLAG, MCH):
            emit_b(mb)
        bt_sb = pp.tile([P, P], BF16, tag="bt_sb")
        nc.vector.tensor_copy(bt_sb[:], b_ps[:])

        # ---- phase 2: fold the y stage into the W projection.  Since y is
        # only consumed by W, wy = Ww y = Ww B^T theta = D^T theta with
        # D = B^T-acc @ WwT  [128, 256] -- one tiny matmul -- and the whole
        # former y phase (8 matmuls + 8 drains) disappears.  bn_stats on DVE
        # reads the W PSUM directly so the stats path does not serialize
        # behind the ACT wy drains (those are only needed for the affine).
        d_ps = ps_tr.tile([P, 512], F32, tag="tr", name="d_ps")
        nc.tensor.matmul(d_ps[:, 0:256], bt_sb[:], wp_t[:, 768:1024],
                         start=True, stop=True, skip_group_check=True)
        d_sb = pp.tile([P, 256], BF16, tag="d_sb")
        nc.vector.tensor_copy(d_sb[:], d_ps[:, 0:256])

        for j in range(NCH):
            for h in range(2):
                w_ps = ps_cv.tile([P, 512], F32, tag="cv", name="ps_w")
                nc.tensor.matmul(w_ps[:], d_sb[:, cs(h, P)],
                                 th_t[:, cs(j, 512)], start=True, stop=True)
                nc.vector.bn_stats(bst_t[h][:, cs(j, 6)], w_ps[:])
                nc.scalar.activation(wy_t[h][:, cs(j, 512)], w_ps[:], AF.Copy)

        # ---- BN stats: bn_aggr per half -> sums, AllReduce, global affine
        mv = pp.tile([P, 4], F32, tag="mv")          # [m0 v0 | m1 v1]
        for h in range(2):
            nc.vector.bn_aggr(mv[:, cs(h, 2)], bst_t[h][:, :])
        s4 = pp.tile([P, 4], F32, tag="s4")          # [S1_0 S1_1 S2_0 S2_1]
        msq = pp.tile([P, 2], F32, tag="msq")
        # scaled by 1/B so the post-AllReduce sums are directly the global
        # mean and E[w^2] (the 1/(B*N) normalization is pre-folded here,
        # where there is slack before the collective):
        # S1 = m/B ; S2 = (v + m^2)/B
        invb = 1.0 / B
        # h0 chain on DVE, h1 chain partly on ACT -- shortens the serial
        # stats path between the last W chunk and the collective trigger
        nc.scalar.activation(s4[:, 1:2], mv[:, 2:3], AF.Copy, scale=invb)
        nc.scalar.square(msq[:, 1:2], mv[:, 2:3])
        nc.vector.tensor_scalar_mul(s4[:, 0:1], mv[:, 0:1], invb)
        nc.vector.tensor_tensor(msq[:, 0:1], mv[:, 0:1], mv[:, 0:1], op=ALU.mult)
        nc.vector.tensor_tensor(msq[:, 0:1], msq[:, 0:1], mv[:, 1:2], op=ALU.add)
        nc.vector.tensor_tensor(msq[:, 1:2], msq[:, 1:2], mv[:, 3:4], op=ALU.add)
        nc.vector.tensor_scalar_mul(s4[:, 2:3], msq[:, 0:1], invb)
        nc.vector.tensor_scalar_mul(s4[:, 3:4], msq[:, 1:2], invb)
        rdma_waits = []
        if USE_RDMA:
            part_t = [s4,
                      pp.tile([P, 4], F32, tag="part1", name="part1"),
                      pp.tile([P, 4], F32, tag="part2", name="part2"),
                      pp.tile([P, 4], F32, tag="g4", name="g4")]
            rdests = [
                [(0, 1)] + [None] * 7,                       # XOR 1
                [(0, 2)] + [None] * 7,                       # XOR 2
                [None] * 4 + [(0, 4)] + [None] * 3,          # XOR 4 (D2D slot)
            ]
            # The adds' waits on the remote-landing semaphore are attached
            # AFTER tile scheduling (the single-core scheduling sim cannot
            # model peer increments and would report a deadlock).
            for r in range(RDMA_ROUNDS):
                nc.gpsimd.remote_dma_broadcast(
                    land_t[r][:], part_t[r][:],
                    remote_sem=sem_r, local_sem=sem_l, rdests=rdests[r])
                nc.gpsimd.trigger_dma(count=None)
                add_i = nc.vector.tensor_tensor(part_t[r + 1][:],
                                                part_t[r][:],
                                                land_t[r][:], op=ALU.add)
                rdma_waits.append((add_i, 2 * (r + 1)))
            g4 = part_t[RDMA_ROUNDS]
        elif USE_ALLGATHER:
            in_b = dram.tile([P, 4], F32)
            out_b = dram.tile([P * N_CORES, 4], F32)
            nc.sync.dma_start(in_b[:], s4[:])
            nc.gpsimd.collective_compute(
                "AllGather", ALU.bypass,
                replica_groups=[list(range(N_CORES))],
                ins=[in_b.opt()], outs=[out_b.opt()],
            )
            # readback as 8 contiguous [128,4] block loads (the single
            # strided gather costs ~6us in descriptor processing)
            g32 = pp.tile([P, 32], F32, tag="g32")
            rb_eng = [nc.sync, nc.scalar, nc.sync, nc.scalar,
                      nc.sync, nc.scalar, nc.sync, nc.scalar]
            for r in range(N_CORES):
                rb_eng[r].dma_start(g32[:, cs(r, 4)], out_b[r * P:(r + 1) * P, :])
            g4 = pp.tile([P, 4], F32, tag="g4", name="g4")
            nc.vector.reduce_sum(g4[:], g32[:].rearrange("p (r c) -> p c r",
                                                         r=N_CORES), axis=AX.X)
        else:
            in_b = dram.tile([P, 4], F32)
            out_b = dram.tile([P, 4], F32)
            nc.sync.dma_start(in_b[:], s4[:])
            nc.gpsimd.collective_compute(
                "AllReduce", ALU.add,
                replica_groups=[list(range(N_CORES))],
                ins=[in_b.opt()], outs=[out_b.opt()],
            )
            g4 = pp.tile([P, 4], F32, tag="g4", name="g4")
            nc.sync.dma_start(g4[:], out_b[:])

        var = pp.tile([P, 2], F32, tag="var")
        tmp = pp.tile([P, 2], F32, tag="tmp")
        sd = pp.tile([P, 2], F32, tag="sd")
        scl = pp.tile([P, 2], F32, tag="scl")
        bia = pp.tile([P, 2], F32, tag="bia")
        mn = g4[:, 0:2]                          # global mean (pre-scaled)
        nc.vector.tensor_mul(tmp[:], mn, mn)
        nc.vector.tensor_sub(var[:], g4[:, 2:4], tmp[:])
        nc.scalar.activation(sd[:], var[:], AF.Sqrt, bias=eps_t[:, 0:1])
        rstd = pp.tile([P, 2], F32, tag="rstd")
        nc.vector.reciprocal(rstd[:], sd[:])
        nc.vector.tensor_mul(scl[:], rstd[:], gam_t)
        nc.vector.tensor_mul(tmp[:], mn, scl[:])
        nc.vector.tensor_sub(bia[:], bet_t, tmp[:])

        # ---- normalize + residual + store (all bf16): [128,2048] chunks to
        # amortize per-op fixed costs; affine split ACT/DVE (3:1), adds on
        # DVE, each chunk stored as two [128,1024] halves on both HWDGE
        # rings.  This tail is the only stats-dependent work, so it alone
        # sits after the AllReduce.
        for idx in range(4):
            h, j = divmod(idx, 2)
            o1 = op.tile([P, 2048], BF16, tag="o1", name="o1")
            o2 = op.tile([P, 2048], BF16, tag="o2", name="o2")
            if idx < 3:
                nc.scalar.activation(o1[:], wy_t[h][:, cs(j, 2048)],
                                     AF.Identity, bias=bia[:, h:h + 1],
                                     scale=scl[:, h:h + 1])
            else:
                nc.vector.tensor_scalar(o1[:], wy_t[h][:, cs(j, 2048)],
                                        scl[:, h:h + 1], bia[:, h:h + 1],
                                        op0=ALU.mult, op1=ALU.add)
            nc.vector.tensor_tensor(o2[:], o1[:], x16h[h][:, cs(j, 2048)],
                                    op=ALU.add)
            nc.sync.dma_start(out16[h * P:(h + 1) * P,
                                    2048 * j:2048 * j + 1024], o2[:, 0:1024])
            nc.scalar.dma_start(out16[h * P:(h + 1) * P,
                                      2048 * j + 1024:2048 * (j + 1)],
                                o2[:, 1024:2048])

    for add_i, thr in rdma_waits:
        add_i.wait_op(sem_r, thr, "sem-ge", check=False)
    nc.compile()
    return nc


_CACHE = {}


def _get_module():
    if "nc" not in _CACHE:
        _CACHE["nc"] = _build_module()
    return _CACHE["nc"]


def _prep_in_maps(x, g_w, g_b, theta_w, theta_b, phi_w, phi_b, W_w, W_b,
                  bn_gamma, bn_beta):
    bf = ml_dtypes.bfloat16
    f32 = np.float32
    thwT = (theta_w.T / N).astype(bf)
    phwT = phi_w.T.astype(bf)
    gwT = g_w.T.astype(bf)
    WwT = W_w.T.astype(bf)
    wpack = np.concatenate(
        [thwT[0:P], thwT[P:2 * P],
         phwT[0:P], gwT[0:P], phwT[P:2 * P], gwT[P:2 * P],
         WwT[:, 0:P], WwT[:, P:2 * P]], axis=1)
    pgb = np.broadcast_to(
        np.concatenate([phi_b, g_b, phi_b, g_b])[None, :], (P, 512))
    bpack = np.concatenate(
        [(theta_b / N).reshape(P, 1).astype(f32),
         bn_gamma.reshape(2, P).T.astype(f32),
         bn_beta.reshape(2, P).T.astype(f32),
         pgb.astype(f32)], axis=1)
    shared = {
        "wpack": np.ascontiguousarray(wpack),
        "bpack": np.ascontiguousarray(bpack),
    }
    x16 = np.ascontiguousarray(x.astype(bf))
    in_maps = []
    for i in range(N_CORES):
        m = dict(shared)
        m["x16"] = x16[i]
        in_maps.append(m)
    return in_maps


def _run(inputs, trace=False, trace_cores=None):
    nc = _get_module()
    in_maps = _prep_in_maps(**inputs)
    res = bass_utils.run_bass_kernel_spmd(
        nc, in_maps, core_ids=list(range(N_CORES)),
        trace=trace, trace_cores=trace_cores,
    )
    out = np.stack([res.results[i]["out16"] for i in range(N_CORES)], axis=0)
    return out.astype(np.float32), res


def kernel(**inputs) -> np.ndarray:
    out, _ = _run(inputs, trace=False)
    return out
